# revision 5
# baseline (speedup 1.0000x reference)
"""Trainium2 Bass kernel for nn_Classifier (attention-pool + linear + classifier).

Reference math (per state n of 64):
    attn  = softmax(output_set @ states[n].T, axis=-1)      # [64io, 512s]
    mix   = attn @ states[n]                                # [64io, 1024h]
    o     = [mix | output_set] @ Wo + bo                    # [64io, 1024h]
    logit = tanh(o).flatten() @ Wc + bc                     # [64]

Sharding: data-parallel over the leading n_states dim — 8 states per core on
8 cores. Each core computes its own [8, 64] logits slice; host concatenates.

Schedule (per core):
  - states processed in PAIRS packed into the 128-partition dim; quadrant
    (tile_position) matmuls stream both states' attn/mix concurrently.
  - statesT (score path) is fp8: the softmax only sees scores, so fp8
    quantization of the score operands is harmless; the value path (states
    s-major for mix) stays bf16. The const matmul keeps a bf16 osT copy
    (PE matmuls need matching operand dtypes).
  - scores are bounded (|s| < 1), so softmax skips the max-subtraction:
    exp straight off the score PSUM with a fused sum (accum_out).
  - DMA emission order prioritizes the data the pipeline needs next;
    pair-granular merged transfers keep the descriptor count low; Wc is
    prefetched into resident SBUF during the pair loop so the classifier
    never waits on HBM.
  - pair stages are software-pipelined: pair i's attn matmuls are emitted
    before pair i-1's post-softmax stages so the PE fills softmax gaps.
  - epilogue folds the classifier's odd/even PSUM quadrants with a tiny
    stacked-identity matmul instead of a gpsimd accumulate DMA.
"""

import os
import sys

import numpy as np

for _p in ("/opt/trn_rl_repo",):
    if _p not in sys.path:
        sys.path.insert(0, _p)

import ml_dtypes

import concourse.bass as bass
import concourse.mybir as mybir
import concourse.tile as tile
from concourse import bacc
from concourse.masks import make_identity

IO, H, S, NTOT = 64, 1024, 512, 64
NCORES = 8
NLOC = NTOT // NCORES  # states per core
P = 128
HC = H // P  # 8 h-chunks
SC = S // P  # 4 s-chunks
NPAIR = NLOC // 2

DT = mybir.dt.bfloat16
NPDT = ml_dtypes.bfloat16
F8 = mybir.dt.float8e4
NPF8 = ml_dtypes.float8_e4m3

F32 = mybir.dt.float32
AX = mybir.AxisListType
AF = mybir.ActivationFunctionType


def build_bass(reps=1):
    nc = bacc.Bacc(
        "TRN2", target_bir_lowering=False, debug=False, num_devices=NCORES
    )

    statesT_d = nc.declare_dram_parameter("statesT", [NLOC, H, S], F8, isOutput=False)
    states_d = nc.declare_dram_parameter("states", [NLOC, S, H], DT, isOutput=False)
    osT2_d = nc.declare_dram_parameter("osT2", [H, 2 * IO], F8, isOutput=False)
    osT2b_d = nc.declare_dram_parameter("osT2b", [H, 2 * IO], DT, isOutput=False)
    wo_top_d = nc.declare_dram_parameter("wo_top", [H, H], DT, isOutput=False)
    wo_bot_d = nc.declare_dram_parameter("wo_bot", [H, H], DT, isOutput=False)
    bo2_d = nc.declare_dram_parameter("bo2", [P, H], F32, isOutput=False)
    # classifier weights, pair-packed: [hp, j, hc, t*64+c] = Wc[(2j+t)*H + hc*128 + hp, c]
    wc_d = nc.declare_dram_parameter("wc", [P, IO // 2, HC, P], DT, isOutput=False)
    bct_d = nc.declare_dram_parameter("bct", [IO, NLOC], F32, isOutput=False)
    out_d = nc.declare_dram_parameter("logitsT", [IO, NLOC], F32, isOutput=True)

    NWG = 2  # prefetched Wc groups
    NJG = IO // 2 // NWG  # io-pairs per group (16)

    with tile.TileContext(nc) as tc:
        with (
            tc.tile_pool(name="consts", bufs=1) as consts,
            tc.tile_pool(name="stT", bufs=4) as stT_pool,
            tc.tile_pool(name="sn", bufs=2) as sn_pool,
            tc.tile_pool(name="work", bufs=2) as work,
            tc.tile_pool(name="sm", bufs=4) as sm_pool,
            tc.tile_pool(name="ps_attn", bufs=2, space="PSUM") as ps_attn,
            tc.tile_pool(name="ps_tr", bufs=2, space="PSUM") as ps_tr,
            tc.tile_pool(name="ps_mix", bufs=1, space="PSUM") as ps_mix,
            tc.tile_pool(name="ps_o", bufs=1, space="PSUM") as ps_o,
        ):
            # ---- resident tiles ----
            osT2_sb = consts.tile([P, HC, 2 * IO], F8)
            osT2b_sb = consts.tile([P, HC, 2 * IO], DT)
            wo_top_sb = consts.tile([P, HC, H], DT)
            wob_sb = consts.tile([P, HC, H], DT)
            ident = consts.tile([P, P], DT)
            sel = consts.tile([P, IO], DT)
            bo2_sb = consts.tile([P, H], F32)
            bct_sb = consts.tile([IO, NLOC], F32)
            const_sb = consts.tile([P, H], F32)
            # tanh(o) transposed, io-major: [hp, hc, io, state]
            tT_all = consts.tile([P, HC, IO, NLOC], DT)
            # resident classifier weights (prefetched during the pair loop)
            wc_sb = [
                consts.tile([P, NJG, HC, P], DT, name=f"wc_sb{g}")
                for g in range(NWG)
            ]

            for _rep in range(reps):
                stT = {}
                sn = {}

                def issue_stT(pi):
                    stT[pi] = stT_pool.tile(
                        [P, 2, HC, S], F8, tag="stT", name=f"stT_{pi}"
                    )
                    nc.sync.dma_start(
                        stT[pi][:],
                        statesT_d[2 * pi : 2 * pi + 2].rearrange(
                            "n (hc p) s -> p n hc s", p=P
                        ),
                    )

                def issue_sn(pi):
                    sn[pi] = sn_pool.tile(
                        [P, 2, SC, H], DT, tag="sn", name=f"sn_{pi}"
                    )
                    nc.sync.dma_start(
                        sn[pi][:],
                        states_d[2 * pi : 2 * pi + 2].rearrange(
                            "n (sc p) h -> p n sc h", p=P
                        ),
                    )

                aps = {}

                def attn(pi):
                    # attn scores: [128(ioA|ioB), 512s], fp8 operands
                    aps[pi] = ps_attn.tile(
                        [P, S], F32, tag="ps_attn", name=f"aps_{pi}"
                    )
                    for hc in range(HC):
                        for s_i in (0, 1):
                            nc.tensor.matmul(
                                aps[pi][s_i * IO : (s_i + 1) * IO, :],
                                lhsT=osT2_sb[:, hc, s_i * IO : (s_i + 1) * IO],
                                rhs=stT[pi][:, s_i, hc, :],
                                start=(hc == 0),
                                stop=(hc == HC - 1),
                                tile_position=(0, s_i * IO),
                                skip_group_check=True,
                            )

                attn_w = {}

                def softmax(pi):
                    # softmax over s (free axis), both states at once.
                    # |scores| < 1 so exp() needs no max-subtraction.
                    sumexp = sm_pool.tile([P, 1], F32, tag="sumexp")
                    exps = work.tile([P, S], DT, tag="exps")
                    nc.scalar.activation(
                        exps[:], aps[pi][:], AF.Exp, accum_out=sumexp[:]
                    )
                    rinv = sm_pool.tile([P, 1], F32, tag="rinv")
                    nc.vector.reciprocal(rinv[:], sumexp[:])
                    attn_w[pi] = work.tile([P, S], DT, tag="attn_w", name=f"attn_w{pi}")
                    nc.vector.tensor_scalar_mul(attn_w[pi][:], exps[:], rinv[:])

                def rest(pi):
                    # post-softmax stages for pair pi
                    # attn^T via PE transposes: [128s, (ioA|ioB)]
                    atps = ps_tr.tile([P, 512], DT, tag="ps_tr", name=f"atps_{pi}")
                    for sc in range(SC):
                        nc.tensor.transpose(
                            atps[:, sc * P : (sc + 1) * P],
                            attn_w[pi][:, sc * P : (sc + 1) * P],
                            ident[:],
                        )
                    attnT = work.tile([P, SC, P], DT, tag="attnT")
                    for sc in range(SC):
                        nc.vector.tensor_copy(
                            attnT[:, sc, :], atps[:, sc * P : (sc + 1) * P]
                        )

                    # mix = attn @ states: [128(ioA|ioB), 1024h]
                    mps = ps_mix.tile([P, H], F32, tag="ps_mix")
                    for sc in range(SC):
                        for s_i in (0, 1):
                            for hh in range(2):
                                nc.tensor.matmul(
                                    mps[s_i * IO : (s_i + 1) * IO, hh * 512 : (hh + 1) * 512],
                                    lhsT=attnT[:, sc, s_i * IO : (s_i + 1) * IO],
                                    rhs=sn[pi][:, s_i, sc, hh * 512 : (hh + 1) * 512],
                                    start=(sc == 0),
                                    stop=(sc == SC - 1),
                                    tile_position=(0, s_i * IO),
                                    skip_group_check=True,
                                )
                    mix_sb = work.tile([P, H], DT, tag="mix_sb")
                    nc.vector.tensor_copy(mix_sb[:], mps[:])

                    # mix^T via PE transposes: [128h, (ioA|ioB)] per h-chunk
                    mtps = [
                        ps_tr.tile([P, 512], DT, tag="ps_tr", name=f"mtps_{pi}_{j}")
                        for j in range(2)
                    ]
                    for hc in range(HC):
                        nc.tensor.transpose(
                            mtps[hc // 4][:, (hc % 4) * P : (hc % 4 + 1) * P],
                            mix_sb[:, hc * P : (hc + 1) * P],
                            ident[:],
                        )
                    mixT = work.tile([P, HC, P], DT, tag="mixT")
                    for hc in range(HC):
                        # scalar engine: offload PSUM->SBUF copies from DVE
                        nc.scalar.copy(
                            mixT[:, hc, :], mtps[hc // 4][:, (hc % 4) * P : (hc % 4 + 1) * P]
                        )

                    # o = mix @ Wo_top (+const later): [128(ioA|ioB), 1024h]
                    # full M=128 lhsT (both states) -> single rhs stream
                    ops_ = ps_o.tile([P, H], F32, tag="ps_o", name=f"ops_{pi}")
                    for hc in range(HC):
                        for hh in range(2):
                            nc.tensor.matmul(
                                ops_[:, hh * 512 : (hh + 1) * 512],
                                lhsT=mixT[:, hc, :],
                                rhs=wo_top_sb[:, hc, hh * 512 : (hh + 1) * 512],
                                start=(hc == 0),
                                stop=(hc == HC - 1),
                            )
                    osum = work.tile([P, H], F32, tag="osum")
                    nc.vector.tensor_add(osum[:], ops_[:], const_sb[:])
                    t_sb = work.tile([P, H], DT, tag="t_sb")
                    nc.scalar.activation(t_sb[:], osum[:], AF.Tanh)

                    # t^T into the shared classifier operand buffer
                    ttps = [
                        ps_tr.tile([P, 512], DT, tag="ps_tr", name=f"ttps_{pi}_{j}")
                        for j in range(2)
                    ]
                    for hc in range(HC):
                        nc.tensor.transpose(
                            ttps[hc // 4][:, (hc % 4) * P : (hc % 4 + 1) * P],
                            t_sb[:, hc * P : (hc + 1) * P],
                            ident[:],
                        )
                    for hc in range(HC):
                        # transpose-out cols are (state, io); tT_all wants (io, state)
                        src = ttps[hc // 4][:, (hc % 4) * P : (hc % 4 + 1) * P]
                        nc.vector.tensor_copy(
                            tT_all[:, hc, :, 2 * pi : 2 * pi + 2],
                            src.rearrange("p (st io) -> p io st", st=2),
                        )

                # ================= emission schedule =================
                nc.sync.dma_start(osT2_sb[:], osT2_d.rearrange("(hc p) i -> p hc i", p=P))
                nc.sync.dma_start(
                    osT2b_sb[:], osT2b_d.rearrange("(hc p) i -> p hc i", p=P)
                )
                issue_stT(0)
                nc.sync.dma_start(
                    wob_sb[:], wo_bot_d.rearrange("(hc p) h -> p hc h", p=P)
                )
                issue_sn(0)
                nc.sync.dma_start(bo2_sb[:], bo2_d[:])
                make_identity(nc, ident[:])
                nc.vector.tensor_add(sel[:], ident[:, 0:IO], ident[:, IO:P])

                attn(0)

                # const = output_set @ Wo_bot + bo, duplicated on both halves
                cps = ps_o.tile([P, H], F32, tag="ps_o", name="cps")
                for hc in range(HC):
                    for hh in range(2):
                        nc.tensor.matmul(
                            cps[:, hh * 512 : (hh + 1) * 512],
                            lhsT=osT2b_sb[:, hc, :],
                            rhs=wob_sb[:, hc, hh * 512 : (hh + 1) * 512],
                            start=(hc == 0),
                            stop=(hc == HC - 1),
                        )
                nc.vector.tensor_copy(const_sb[:], cps[:])
                nc.vector.tensor_add(const_sb[:], const_sb[:], bo2_sb[:])

                issue_stT(1)
                nc.sync.dma_start(
                    wo_top_sb[:], wo_top_d.rearrange("(hc p) h -> p hc h", p=P)
                )
                softmax(0)
                issue_sn(1)

                attn(1)
                nc.sync.dma_start(bct_sb[:], bct_d[:])
                rest(0)
                softmax(1)

                issue_stT(2)
                issue_sn(2)
                issue_stT(3)
                nc.sync.dma_start(wc_sb[0][:], wc_d[:, 0:NJG])
                attn(2)
                rest(1)
                softmax(2)

                issue_sn(3)
                nc.sync.dma_start(wc_sb[1][:], wc_d[:, NJG : 2 * NJG])
                attn(3)
                rest(2)
                softmax(3)
                rest(3)

                # ---- classifier, i-pair packed (valid quadrants disjoint in PSUM):
                # lhsT = [Wc_{2j} | Wc_{2j+1}] (128 cols), rhs = [t_{2j} | t_{2j+1}]
                # psum rows 0:64 accumulate even-i partial logitsT, 64:128 odd-i.
                lgps = ps_attn.tile([P, 2 * NLOC], F32, tag="ps_attn", name="lgps")
                for jg in range(NWG):
                    for jl in range(NJG):
                        j = jg * NJG + jl
                        for hc in range(HC):
                            nc.tensor.matmul(
                                lgps[:],
                                lhsT=wc_sb[jg][:, jl, hc, :],
                                rhs=tT_all[:, hc, 2 * j : 2 * j + 2, :],
                                start=(j == 0 and hc == 0),
                                stop=(j == IO // 2 - 1 and hc == HC - 1),
                                skip_group_check=True,
                            )
                # epilogue: fold odd-i quadrant onto even via stacked-identity matmul
                lt2 = work.tile([P, NLOC], DT, tag="lt2")
                nc.vector.tensor_copy(lt2[0:IO, :], lgps[0:IO, 0:NLOC])
                nc.vector.tensor_copy(lt2[IO:P, :], lgps[IO:P, NLOC : 2 * NLOC])
                foldps = ps_attn.tile(
                    [IO, NLOC], F32, tag="ps_attn", name="foldps"
                )
                nc.tensor.matmul(foldps[:], lhsT=sel[:], rhs=lt2[:], start=True, stop=True)
                lt_sb = work.tile([IO, NLOC], F32, tag="lt_sb")
                nc.vector.tensor_add(lt_sb[:], foldps[:], bct_sb[:])
                nc.sync.dma_start(out_d[:], lt_sb[:])

    nc.compile()
    return nc


def make_in_maps(states, output_set, Wo, bo, Wc, bc):
    """Build the per-core input maps (host-side sharding + layout prep)."""
    states = np.asarray(states, dtype=np.float32)
    output_set = np.asarray(output_set, dtype=np.float32)
    Wo = np.asarray(Wo, dtype=np.float32)
    bo = np.asarray(bo, dtype=np.float32)
    Wc = np.asarray(Wc, dtype=np.float32)
    bc = np.asarray(bc, dtype=np.float32)

    osT = output_set.T  # [H, IO]
    osT2 = np.ascontiguousarray(np.concatenate([osT, osT], axis=1))
    shared = {
        "osT2": osT2.astype(NPF8),
        "osT2b": osT2.astype(NPDT),
        "wo_top": np.ascontiguousarray(Wo[:H]).astype(NPDT),
        "wo_bot": np.ascontiguousarray(Wo[H:]).astype(NPDT),
        "bo2": np.ascontiguousarray(np.tile(bo, (P, 1))).astype(np.float32),
        # Wc[(2j+t)*H + hc*128 + hp, c] -> [hp, j, hc, t*64+c]
        "wc": np.ascontiguousarray(
            Wc.reshape(IO // 2, 2, HC, P, IO)
            .transpose(3, 0, 2, 1, 4)
            .reshape(P, IO // 2, HC, P)
        ).astype(NPDT),
        "bct": np.ascontiguousarray(np.tile(bc[:, None], (1, NLOC))).astype(
            np.float32
        ),
    }
    in_maps = []
    for k in range(NCORES):
        sl = states[k * NLOC : (k + 1) * NLOC]  # [NLOC, S, H]
        in_maps.append(
            {
                "states": np.ascontiguousarray(sl).astype(NPDT),
                "statesT": np.ascontiguousarray(sl.transpose(0, 2, 1)).astype(NPF8),
                **shared,
            }
        )
    return in_maps


_NC_CACHE = {}


def get_nc(reps=1):
    if reps not in _NC_CACHE:
        _NC_CACHE[reps] = build_bass(reps)
    return _NC_CACHE[reps]


def kernel(states, output_set, Wo, bo, Wc, bc):
    from concourse.bass_utils import run_bass_kernel_spmd

    nc = get_nc()
    in_maps = make_in_maps(states, output_set, Wo, bo, Wc, bc)
    res = run_bass_kernel_spmd(nc, in_maps, core_ids=list(range(NCORES)))
    out = np.concatenate(
        [np.asarray(res.results[k]["logitsT"]).T for k in range(NCORES)], axis=0
    )
    return out.astype(np.float32)


# revision 6
# speedup vs baseline: 1.0833x; 1.0833x over previous
"""Trainium2 Bass kernel for nn_Classifier (attention-pool + linear + classifier).

Reference math (per state n of 64):
    attn  = softmax(output_set @ states[n].T, axis=-1)      # [64io, 512s]
    mix   = attn @ states[n]                                # [64io, 1024h]
    o     = [mix | output_set] @ Wo + bo                    # [64io, 1024h]
    logit = tanh(o).flatten() @ Wc + bc                     # [64]

Sharding: data-parallel over the leading n_states dim — 8 states per core on
8 cores. Each core computes its own [8, 64] logits slice; host concatenates.

Schedule (per core):
  - states processed in PAIRS packed into the 128-partition dim; quadrant
    (tile_position) matmuls stream both states' scores concurrently.
  - statesT (score path) is fp8: the softmax only sees scores, so fp8
    quantization of the score operands is harmless; the value path (states
    s-major for mix) stays bf16. The const matmul keeps a bf16 osT copy
    (PE matmuls need matching operand dtypes).
  - scores are bounded (|s| < 1), so softmax skips the max-subtraction:
    exp straight off the score PSUM with a fused sum (accum_out).
  - mix is computed FLIPPED (states tiles stationary, attnT moving) so the
    result lands as mixT [h, io] — no PSUM bounce + PE transpose chain.
  - o is computed FLIPPED (Wo tiles stationary, mixT moving) so the result
    lands as oT [h', (state,io)]; tanh reads it with a per-partition bo
    bias and writes the classifier operand layout directly.
  - all DRAM params are host-packed partition-major so every DMA moves
    contiguous multi-KB lines per partition at full HBM rate.
  - Wc is prefetched into resident SBUF during the pair loop so the
    classifier never waits on HBM.
  - epilogue folds the classifier's odd/even PSUM quadrants with a tiny
    stacked-identity matmul instead of a gpsimd accumulate DMA.
"""

import os
import sys

import numpy as np

for _p in ("/opt/trn_rl_repo",):
    if _p not in sys.path:
        sys.path.insert(0, _p)

import ml_dtypes

import concourse.bass as bass
import concourse.mybir as mybir
import concourse.tile as tile
from concourse import bacc
from concourse.masks import make_identity

IO, H, S, NTOT = 64, 1024, 512, 64
NCORES = 8
NLOC = NTOT // NCORES  # states per core
P = 128
HC = H // P  # 8 h-chunks
SC = S // P  # 4 s-chunks
NPAIR = NLOC // 2

DT = mybir.dt.bfloat16
NPDT = ml_dtypes.bfloat16
F8 = mybir.dt.float8e4
NPF8 = ml_dtypes.float8_e4m3

F32 = mybir.dt.float32
AX = mybir.AxisListType
AF = mybir.ActivationFunctionType


def build_bass(reps=1):
    nc = bacc.Bacc(
        "TRN2", target_bir_lowering=False, debug=False, num_devices=NCORES
    )

    # all params partition-major: leading (or post-pair) dim is the SBUF partition
    statesT_d = nc.declare_dram_parameter(
        "statesT", [NPAIR, P, 2, HC, S], F8, isOutput=False
    )
    states_d = nc.declare_dram_parameter(
        "states", [NPAIR, P, 2, SC, H], DT, isOutput=False
    )
    osT2_d = nc.declare_dram_parameter("osT2", [P, HC, 2 * IO], F8, isOutput=False)
    osT2b_d = nc.declare_dram_parameter("osT2b", [P, HC, 2 * IO], DT, isOutput=False)
    wo_top_d = nc.declare_dram_parameter("wo_top", [P, HC, H], DT, isOutput=False)
    wo_bot_d = nc.declare_dram_parameter("wo_bot", [P, HC, H], DT, isOutput=False)
    boT_d = nc.declare_dram_parameter("boT", [P, HC], F32, isOutput=False)
    # classifier weights, pair-packed: [hp, j, hc, t*64+c] = Wc[(2j+t)*H + hc*128 + hp, c]
    wc_d = nc.declare_dram_parameter("wc", [P, IO // 2, HC, P], DT, isOutput=False)
    bct_d = nc.declare_dram_parameter("bct", [IO, NLOC], F32, isOutput=False)
    out_d = nc.declare_dram_parameter("logitsT", [IO, NLOC], F32, isOutput=True)

    NWG = 2  # prefetched Wc groups
    NJG = IO // 2 // NWG  # io-pairs per group (16)

    with tile.TileContext(nc) as tc:
        with (
            tc.tile_pool(name="consts", bufs=1) as consts,
            tc.tile_pool(name="stT", bufs=4) as stT_pool,
            tc.tile_pool(name="sn", bufs=2) as sn_pool,
            tc.tile_pool(name="work", bufs=2) as work,
            tc.tile_pool(name="sm", bufs=4) as sm_pool,
            tc.tile_pool(name="ps_attn", bufs=2, space="PSUM") as ps_attn,
            tc.tile_pool(name="ps_tr", bufs=2, space="PSUM") as ps_tr,
            tc.tile_pool(name="ps_mix", bufs=1, space="PSUM") as ps_mix,
            tc.tile_pool(name="ps_o", bufs=1, space="PSUM") as ps_o,
        ):
            # ---- resident tiles ----
            osT2_sb = consts.tile([P, HC, 2 * IO], F8)
            osT2b_sb = consts.tile([P, HC, 2 * IO], DT)
            wo_top_sb = consts.tile([P, HC, H], DT)
            wob_sb = consts.tile([P, HC, H], DT)
            ident = consts.tile([P, P], DT)
            sel = consts.tile([P, IO], DT)
            boT_sb = consts.tile([P, HC], F32)
            bct_sb = consts.tile([IO, NLOC], F32)
            # os @ Wo_bot transposed, (st,io)-duplicated: [h'p, h'c, 128]
            constT2 = consts.tile([P, HC, P], DT)
            # tanh(o) transposed, io-major: [hp, hc, io, state]
            tT_all = consts.tile([P, HC, IO, NLOC], DT)
            # resident classifier weights (prefetched during the pair loop)
            wc_sb = [
                consts.tile([P, NJG, HC, P], DT, name=f"wc_sb{g}")
                for g in range(NWG)
            ]

            for _rep in range(reps):
                stT = {}
                sn = {}

                def issue_stT(pi):
                    stT[pi] = stT_pool.tile(
                        [P, 2, HC, S], F8, tag="stT", name=f"stT_{pi}"
                    )
                    nc.sync.dma_start(stT[pi][:], statesT_d[pi])

                def issue_sn(pi):
                    sn[pi] = sn_pool.tile(
                        [P, 2, SC, H], DT, tag="sn", name=f"sn_{pi}"
                    )
                    nc.sync.dma_start(sn[pi][:], states_d[pi])

                aps = {}

                def attn(pi):
                    # attn scores: [128(ioA|ioB), 512s], fp8 operands
                    aps[pi] = ps_attn.tile(
                        [P, S], F32, tag="ps_attn", name=f"aps_{pi}"
                    )
                    for hc in range(HC):
                        for s_i in (0, 1):
                            nc.tensor.matmul(
                                aps[pi][s_i * IO : (s_i + 1) * IO, :],
                                lhsT=osT2_sb[:, hc, s_i * IO : (s_i + 1) * IO],
                                rhs=stT[pi][:, s_i, hc, :],
                                start=(hc == 0),
                                stop=(hc == HC - 1),
                                tile_position=(0, s_i * IO),
                                skip_group_check=True,
                            )

                attn_w = {}

                def softmax(pi):
                    # softmax over s (free axis), both states at once.
                    # |scores| < 1 so exp() needs no max-subtraction.
                    sumexp = sm_pool.tile([P, 1], F32, tag="sumexp")
                    exps = work.tile([P, S], DT, tag="exps")
                    nc.scalar.activation(
                        exps[:], aps[pi][:], AF.Exp, accum_out=sumexp[:]
                    )
                    rinv = sm_pool.tile([P, 1], F32, tag="rinv")
                    nc.vector.reciprocal(rinv[:], sumexp[:])
                    attn_w[pi] = work.tile([P, S], DT, tag="attn_w", name=f"attn_w{pi}")
                    nc.vector.tensor_scalar_mul(attn_w[pi][:], exps[:], rinv[:])

                def rest(pi):
                    # post-softmax stages for pair pi
                    # attn^T via PE transposes: [128s, (ioA|ioB)]
                    atps = ps_tr.tile([P, 512], DT, tag="ps_tr", name=f"atps_{pi}")
                    for sc in range(SC):
                        nc.tensor.transpose(
                            atps[:, sc * P : (sc + 1) * P],
                            attn_w[pi][:, sc * P : (sc + 1) * P],
                            ident[:],
                        )
                    attnT = work.tile([P, SC, P], DT, tag="attnT")
                    for sc in range(SC):
                        nc.vector.tensor_copy(
                            attnT[:, sc, :], atps[:, sc * P : (sc + 1) * P]
                        )

                    # flipped mix: states tiles stationary, attnT moving.
                    # mixT[h, io] = sum_s states[s, h] * attn[io, s]
                    # groups g = s_i*HC + ht accumulate over sc.
                    mps = ps_mix.tile([P, 2, HC, IO], F32, tag="ps_mix")
                    for ht in range(HC):
                        for s_i in (0, 1):
                            for sc in range(SC):
                                nc.tensor.matmul(
                                    mps[:, s_i, ht, :],
                                    lhsT=sn[pi][:, s_i, sc, ht * P : (ht + 1) * P],
                                    rhs=attnT[:, sc, s_i * IO : (s_i + 1) * IO],
                                    start=(sc == 0),
                                    stop=(sc == SC - 1),
                                    skip_group_check=True,
                                )
                    # assemble mixT [h, (stA io | stB io)]; split copies DVE/ACT
                    mixT = work.tile([P, HC, P], DT, tag="mixT")
                    for ht in range(HC):
                        nc.vector.tensor_copy(
                            mixT[:, ht, 0:IO], mps[:, 0, ht, :]
                        )
                        nc.scalar.copy(
                            mixT[:, ht, IO:P], mps[:, 1, ht, :]
                        )

                    # flipped o: Wo tiles stationary, mixT moving.
                    # oT[h', (st,io)] = sum_h Wo_top[h, h'] * mixT[h, (st,io)]
                    ops_ = ps_o.tile([P, HC, P], F32, tag="ps_o", name=f"ops_{pi}")
                    for ht in range(HC):
                        for hc in range(HC):
                            nc.tensor.matmul(
                                ops_[:, ht, :],
                                lhsT=wo_top_sb[:, hc, ht * P : (ht + 1) * P],
                                rhs=mixT[:, hc, :],
                                start=(hc == 0),
                                stop=(hc == HC - 1),
                                skip_group_check=True,
                            )
                    # + const (os @ Wo_bot, transposed+duplicated), bf16 out
                    osumT = work.tile([P, HC, P], DT, tag="osumT")
                    nc.vector.tensor_add(osumT[:], ops_[:], constT2[:])
                    # tanh with per-partition bo bias, writing the classifier
                    # layout directly: tT_all[:, ht, io, state]
                    for ht in range(HC):
                        nc.scalar.activation(
                            tT_all[:, ht, :, 2 * pi : 2 * pi + 2],
                            osumT[:, ht, :].rearrange("p (st io) -> p io st", st=2),
                            AF.Tanh,
                            bias=boT_sb[:, ht : ht + 1],
                        )

                # ================= emission schedule =================
                nc.sync.dma_start(osT2_sb[:], osT2_d[:])
                nc.sync.dma_start(osT2b_sb[:], osT2b_d[:])
                issue_stT(0)
                nc.sync.dma_start(wob_sb[:], wo_bot_d[:])
                issue_sn(0)
                nc.sync.dma_start(boT_sb[:], boT_d[:])
                make_identity(nc, ident[:])
                nc.vector.tensor_add(sel[:], ident[:, 0:IO], ident[:, IO:P])

                attn(0)

                # constT = (output_set @ Wo_bot)^T, flipped like o; no bias
                cps = ps_o.tile([P, HC, IO], F32, tag="ps_o", name="cps")
                for ht in range(HC):
                    for hc in range(HC):
                        nc.tensor.matmul(
                            cps[:, ht, :],
                            lhsT=wob_sb[:, hc, ht * P : (ht + 1) * P],
                            rhs=osT2b_sb[:, hc, 0:IO],
                            start=(hc == 0),
                            stop=(hc == HC - 1),
                            skip_group_check=True,
                        )
                nc.vector.tensor_copy(constT2[:, :, 0:IO], cps[:])
                nc.scalar.copy(constT2[:, :, IO:P], cps[:])

                issue_stT(1)
                nc.sync.dma_start(wo_top_sb[:], wo_top_d[:])
                softmax(0)
                issue_sn(1)

                attn(1)
                nc.sync.dma_start(bct_sb[:], bct_d[:])
                rest(0)
                softmax(1)

                issue_stT(2)
                issue_sn(2)
                issue_stT(3)
                nc.sync.dma_start(wc_sb[0][:], wc_d[:, 0:NJG])
                attn(2)
                rest(1)
                softmax(2)

                issue_sn(3)
                nc.sync.dma_start(wc_sb[1][:], wc_d[:, NJG : 2 * NJG])
                attn(3)
                rest(2)
                softmax(3)
                rest(3)

                # ---- classifier, i-pair packed (valid quadrants disjoint in PSUM):
                # lhsT = [Wc_{2j} | Wc_{2j+1}] (128 cols), rhs = [t_{2j} | t_{2j+1}]
                # psum rows 0:64 accumulate even-i partial logitsT, 64:128 odd-i.
                lgps = ps_attn.tile([P, 2 * NLOC], F32, tag="ps_attn", name="lgps")
                for jg in range(NWG):
                    for jl in range(NJG):
                        j = jg * NJG + jl
                        for hc in range(HC):
                            nc.tensor.matmul(
                                lgps[:],
                                lhsT=wc_sb[jg][:, jl, hc, :],
                                rhs=tT_all[:, hc, 2 * j : 2 * j + 2, :],
                                start=(j == 0 and hc == 0),
                                stop=(j == IO // 2 - 1 and hc == HC - 1),
                                skip_group_check=True,
                            )
                # epilogue: fold odd-i quadrant onto even via stacked-identity matmul
                lt2 = work.tile([P, NLOC], DT, tag="lt2")
                nc.vector.tensor_copy(lt2[0:IO, :], lgps[0:IO, 0:NLOC])
                nc.vector.tensor_copy(lt2[IO:P, :], lgps[IO:P, NLOC : 2 * NLOC])
                foldps = ps_attn.tile(
                    [IO, NLOC], F32, tag="ps_attn", name="foldps"
                )
                nc.tensor.matmul(foldps[:], lhsT=sel[:], rhs=lt2[:], start=True, stop=True)
                lt_sb = work.tile([IO, NLOC], F32, tag="lt_sb")
                nc.vector.tensor_add(lt_sb[:], foldps[:], bct_sb[:])
                nc.sync.dma_start(out_d[:], lt_sb[:])

    nc.compile()
    return nc


def make_in_maps(states, output_set, Wo, bo, Wc, bc):
    """Build the per-core input maps (host-side sharding + layout prep)."""
    states = np.asarray(states, dtype=np.float32)
    output_set = np.asarray(output_set, dtype=np.float32)
    Wo = np.asarray(Wo, dtype=np.float32)
    bo = np.asarray(bo, dtype=np.float32)
    Wc = np.asarray(Wc, dtype=np.float32)
    bc = np.asarray(bc, dtype=np.float32)

    osT = output_set.T  # [H, IO]
    osT2 = np.concatenate([osT, osT], axis=1).reshape(HC, P, 2 * IO).transpose(1, 0, 2)
    shared = {
        "osT2": np.ascontiguousarray(osT2).astype(NPF8),
        "osT2b": np.ascontiguousarray(osT2).astype(NPDT),
        "wo_top": np.ascontiguousarray(
            Wo[:H].reshape(HC, P, H).transpose(1, 0, 2)
        ).astype(NPDT),
        "wo_bot": np.ascontiguousarray(
            Wo[H:].reshape(HC, P, H).transpose(1, 0, 2)
        ).astype(NPDT),
        "boT": np.ascontiguousarray(bo.reshape(HC, P).T).astype(np.float32),
        # Wc[(2j+t)*H + hc*128 + hp, c] -> [hp, j, hc, t*64+c]
        "wc": np.ascontiguousarray(
            Wc.reshape(IO // 2, 2, HC, P, IO)
            .transpose(3, 0, 2, 1, 4)
            .reshape(P, IO // 2, HC, P)
        ).astype(NPDT),
        "bct": np.ascontiguousarray(np.tile(bc[:, None], (1, NLOC))).astype(
            np.float32
        ),
    }
    in_maps = []
    for k in range(NCORES):
        sl = states[k * NLOC : (k + 1) * NLOC]  # [NLOC, S, H]
        # [NPAIR, P, 2, SC, H]: [pi, p, n, sc, h] = sl[2pi+n, sc*128+p, h]
        sn_pack = sl.reshape(NPAIR, 2, SC, P, H).transpose(0, 3, 1, 2, 4)
        # [NPAIR, P, 2, HC, S]: [pi, p, n, hc, s] = sl[2pi+n, s, hc*128+p]
        stT_pack = sl.reshape(NPAIR, 2, S, HC, P).transpose(0, 4, 1, 3, 2)
        in_maps.append(
            {
                "states": np.ascontiguousarray(sn_pack).astype(NPDT),
                "statesT": np.ascontiguousarray(stT_pack).astype(NPF8),
                **shared,
            }
        )
    return in_maps


_NC_CACHE = {}


def get_nc(reps=1):
    if reps not in _NC_CACHE:
        _NC_CACHE[reps] = build_bass(reps)
    return _NC_CACHE[reps]


def kernel(states, output_set, Wo, bo, Wc, bc):
    from concourse.bass_utils import run_bass_kernel_spmd

    nc = get_nc()
    in_maps = make_in_maps(states, output_set, Wo, bo, Wc, bc)
    res = run_bass_kernel_spmd(nc, in_maps, core_ids=list(range(NCORES)))
    out = np.concatenate(
        [np.asarray(res.results[k]["logitsT"]).T for k in range(NCORES)], axis=0
    )
    return out.astype(np.float32)


# revision 10
# speedup vs baseline: 1.0968x; 1.0124x over previous
"""Trainium2 Bass kernel for nn_Classifier (attention-pool + linear + classifier).

Reference math (per state n of 64):
    attn  = softmax(output_set @ states[n].T, axis=-1)      # [64io, 512s]
    mix   = attn @ states[n]                                # [64io, 1024h]
    o     = [mix | output_set] @ Wo + bo                    # [64io, 1024h]
    logit = tanh(o).flatten() @ Wc + bc                     # [64]

Sharding: data-parallel over the leading n_states dim — 8 states per core on
8 cores. Each core computes its own [8, 64] logits slice; host concatenates.

Schedule (per core):
  - states processed in PAIRS packed into the 128-partition dim; quadrant
    (tile_position) matmuls stream both states' scores concurrently, in fp8
    DoubleRow mode (two K-tiles per pass).
  - statesT (score path) is fp8: the softmax only sees scores, so fp8
    quantization of the score operands is harmless; the value path (states
    s-major for mix) stays bf16.
  - scores are bounded (|s| < 1), so softmax skips the max-subtraction:
    exp straight off the score PSUM with a fused sum (accum_out).
  - mix is computed FLIPPED (states tiles stationary, attnT moving) so the
    result lands as mixT [h, io] — no PSUM bounce + PE transpose chain.
  - o is computed FLIPPED (Wo tiles stationary, mixT moving) so the result
    lands as oT [h', (state,io)]; tanh reads it with a per-partition bo
    bias and writes the classifier operand layout directly.
  - pair stages are software-pipelined: pair i's score matmuls are emitted
    in two halves inside pair i-1's post-softmax stages so the PE keeps
    busy across the cross-engine dependency points; PSUM->SBUF copies are
    split across DVE and ACT to halve their latency.
  - all DRAM params are host-packed partition-major so every DMA moves
    contiguous multi-KB lines per partition at full HBM rate; Wc is
    prefetched into resident SBUF during the pair loop.
  - the classifier iterates hc-outer so its first matmuls only depend on
    the first tanh slice of the last pair (subtile deps overlap the tail);
    the odd/even PSUM quadrant fold is a tiny stacked-identity matmul.
"""

import os
import sys

import numpy as np

for _p in ("/opt/trn_rl_repo",):
    if _p not in sys.path:
        sys.path.insert(0, _p)

import ml_dtypes

import concourse.bass as bass
import concourse.mybir as mybir
import concourse.tile as tile
from concourse import bacc
from concourse.masks import make_identity

IO, H, S, NTOT = 64, 1024, 512, 64
NCORES = 8
NLOC = NTOT // NCORES  # states per core
P = 128
HC = H // P  # 8 h-chunks
SC = S // P  # 4 s-chunks
NPAIR = NLOC // 2

DT = mybir.dt.bfloat16
NPDT = ml_dtypes.bfloat16
F8 = mybir.dt.float8e4
NPF8 = ml_dtypes.float8_e4m3

F32 = mybir.dt.float32
AX = mybir.AxisListType
AF = mybir.ActivationFunctionType
DR = mybir.MatmulPerfMode.DoubleRow


def build_bass(reps=1):
    nc = bacc.Bacc(
        "TRN2", target_bir_lowering=False, debug=False, num_devices=NCORES
    )

    # all params partition-major: leading (or post-pair) dim is the SBUF partition
    statesT_d = nc.declare_dram_parameter(
        "statesT", [NPAIR, P, 2, HC, S], F8, isOutput=False
    )
    states_d = nc.declare_dram_parameter(
        "states", [NPAIR, P, 2, SC, H], DT, isOutput=False
    )
    osT2_d = nc.declare_dram_parameter("osT2", [P, HC, 2 * IO], F8, isOutput=False)
    osT2b_d = nc.declare_dram_parameter("osT2b", [P, HC, 2 * IO], DT, isOutput=False)
    wo_top_d = nc.declare_dram_parameter("wo_top", [P, HC, H], DT, isOutput=False)
    wo_bot_d = nc.declare_dram_parameter("wo_bot", [P, HC, H], DT, isOutput=False)
    boT_d = nc.declare_dram_parameter("boT", [P, HC], F32, isOutput=False)
    # classifier weights, pair-packed: [hp, j, hc, t*64+c] = Wc[(2j+t)*H + hc*128 + hp, c]
    wc_d = nc.declare_dram_parameter("wc", [P, IO // 2, HC, P], DT, isOutput=False)
    bct_d = nc.declare_dram_parameter("bct", [IO, NLOC], F32, isOutput=False)
    out_d = nc.declare_dram_parameter("logitsT", [IO, NLOC], F32, isOutput=True)

    NWG = 2  # prefetched Wc groups
    NJG = IO // 2 // NWG  # io-pairs per group (16)

    with tile.TileContext(nc) as tc:
        with (
            tc.tile_pool(name="consts", bufs=1) as consts,
            tc.tile_pool(name="stT", bufs=4) as stT_pool,
            tc.tile_pool(name="sn", bufs=3) as sn_pool,
            tc.tile_pool(name="work", bufs=2) as work,
            tc.tile_pool(name="sm", bufs=4) as sm_pool,
            tc.tile_pool(name="ps_attn", bufs=2, space="PSUM") as ps_attn,
            tc.tile_pool(name="ps_tr", bufs=2, space="PSUM") as ps_tr,
            tc.tile_pool(name="ps_mix", bufs=1, space="PSUM") as ps_mix,
            tc.tile_pool(name="ps_o", bufs=1, space="PSUM") as ps_o,
        ):
            # ---- resident tiles ----
            osT2_sb = consts.tile([P, HC, 2 * IO], F8)
            osT2b_sb = consts.tile([P, HC, 2 * IO], DT)
            wo_top_sb = consts.tile([P, HC, H], DT)
            wob_sb = consts.tile([P, HC, H], DT)
            ident = consts.tile([P, P], DT)
            sel = consts.tile([P, IO], DT)
            boT_sb = consts.tile([P, HC], F32)
            bct_sb = consts.tile([IO, NLOC], F32)
            # os @ Wo_bot transposed, (st,io)-duplicated: [h'p, h'c, 128]
            constT2 = consts.tile([P, HC, P], DT)
            # tanh(o) transposed, io-major: [hp, hc, io, state]
            tT_all = consts.tile([P, HC, IO, NLOC], DT)
            # resident classifier weights (prefetched during the pair loop)
            wc_sb = [
                consts.tile([P, NJG, HC, P], DT, name=f"wc_sb{g}")
                for g in range(NWG)
            ]

            for _rep in range(reps):
                stT = {}
                sn = {}

                def issue_stT(pi):
                    stT[pi] = stT_pool.tile(
                        [P, 2, HC, S], F8, tag="stT", name=f"stT_{pi}"
                    )
                    nc.sync.dma_start(stT[pi][:], statesT_d[pi])

                def issue_sn(pi):
                    sn[pi] = sn_pool.tile(
                        [P, 2, SC, H], DT, tag="sn", name=f"sn_{pi}"
                    )
                    nc.sync.dma_start(sn[pi][:], states_d[pi])

                aps = {}

                USE_DR = os.environ.get("KBASS_DR", "1") == "1"

                def attn_part(pi, qs):
                    # attn scores: [128(ioA|ioB), 512s], fp8 DoubleRow
                    # (two h-chunks contracted per pass)
                    if qs[0] == 0:
                        aps[pi] = ps_attn.tile(
                            [P, S], F32, tag="ps_attn", name=f"aps_{pi}"
                        )
                    for q in qs:
                        for s_i in (0, 1):
                            if USE_DR:
                                nc.tensor.matmul(
                                    aps[pi][s_i * IO : (s_i + 1) * IO, :],
                                    lhsT=osT2_sb[:, 2 * q : 2 * q + 2, s_i * IO : (s_i + 1) * IO],
                                    rhs=stT[pi][:, s_i, 2 * q : 2 * q + 2, :],
                                    start=(q == 0),
                                    stop=(q == HC // 2 - 1),
                                    perf_mode=DR,
                                    tile_position=(0, s_i * IO),
                                    skip_group_check=True,
                                )
                            else:
                                for hq in range(2):
                                    hc = 2 * q + hq
                                    nc.tensor.matmul(
                                        aps[pi][s_i * IO : (s_i + 1) * IO, :],
                                        lhsT=osT2_sb[:, hc, s_i * IO : (s_i + 1) * IO],
                                        rhs=stT[pi][:, s_i, hc, :],
                                        start=(hc == 0),
                                        stop=(hc == HC - 1),
                                        tile_position=(0, s_i * IO),
                                        skip_group_check=True,
                                    )

                attn_w = {}

                def softmax(pi):
                    # softmax over s (free axis), both states at once.
                    # |scores| < 1 so exp() needs no max-subtraction.
                    sumexp = sm_pool.tile([P, 1], F32, tag="sumexp")
                    exps = work.tile([P, S], DT, tag="exps")
                    nc.scalar.activation(
                        exps[:], aps[pi][:], AF.Exp, accum_out=sumexp[:]
                    )
                    rinv = sm_pool.tile([P, 1], F32, tag="rinv")
                    nc.vector.reciprocal(rinv[:], sumexp[:])
                    attn_w[pi] = work.tile([P, S], DT, tag="attn_w", name=f"attn_w{pi}")
                    nc.vector.tensor_scalar_mul(attn_w[pi][:], exps[:], rinv[:])

                atps_t = {}
                attnT_t = {}
                mps_t = {}
                mixT_t = {}

                def atrans(pi):
                    # attn^T via PE transposes: [128s, (ioA|ioB)]
                    atps_t[pi] = ps_tr.tile([P, 512], DT, tag="ps_tr", name=f"atps_{pi}")
                    for sc in range(SC):
                        nc.tensor.transpose(
                            atps_t[pi][:, sc * P : (sc + 1) * P],
                            attn_w[pi][:, sc * P : (sc + 1) * P],
                            ident[:],
                        )

                def attnT_copies(pi):
                    attnT_t[pi] = work.tile([P, SC, P], DT, tag="attnT", name=f"attnT{pi}")
                    for sc in range(SC):
                        if sc % 2 == 0:
                            nc.vector.tensor_copy(
                                attnT_t[pi][:, sc, :], atps_t[pi][:, sc * P : (sc + 1) * P]
                            )
                        else:
                            nc.scalar.copy(
                                attnT_t[pi][:, sc, :], atps_t[pi][:, sc * P : (sc + 1) * P]
                            )

                def flipmix(pi):
                    # flipped mix: states tiles stationary, attnT moving.
                    # mixT[h, io] = sum_s states[s, h] * attn[io, s]
                    mps_t[pi] = ps_mix.tile([P, 2, HC, IO], F32, tag="ps_mix", name=f"mps_{pi}")
                    for ht in range(HC):
                        for s_i in (0, 1):
                            for sc in range(SC):
                                nc.tensor.matmul(
                                    mps_t[pi][:, s_i, ht, :],
                                    lhsT=sn[pi][:, s_i, sc, ht * P : (ht + 1) * P],
                                    rhs=attnT_t[pi][:, sc, s_i * IO : (s_i + 1) * IO],
                                    start=(sc == 0),
                                    stop=(sc == SC - 1),
                                    skip_group_check=True,
                                )

                def mixT_copies(pi):
                    # assemble mixT [h, (stA io | stB io)]; split copies DVE/ACT
                    mixT_t[pi] = work.tile([P, HC, P], DT, tag="mixT", name=f"mixT{pi}")
                    for ht in range(HC):
                        nc.vector.tensor_copy(
                            mixT_t[pi][:, ht, 0:IO], mps_t[pi][:, 0, ht, :]
                        )
                        nc.scalar.copy(
                            mixT_t[pi][:, ht, IO:P], mps_t[pi][:, 1, ht, :]
                        )

                def flipo_tanh(pi):
                    # flipped o: Wo tiles stationary, mixT moving.
                    # oT[h', (st,io)] = sum_h Wo_top[h, h'] * mixT[h, (st,io)]
                    ops_ = ps_o.tile([P, HC, P], F32, tag="ps_o", name=f"ops_{pi}")
                    for ht in range(HC):
                        for hc in range(HC):
                            nc.tensor.matmul(
                                ops_[:, ht, :],
                                lhsT=wo_top_sb[:, hc, ht * P : (ht + 1) * P],
                                rhs=mixT_t[pi][:, hc, :],
                                start=(hc == 0),
                                stop=(hc == HC - 1),
                                skip_group_check=True,
                            )
                    # + const (os @ Wo_bot, transposed+duplicated), bf16 out;
                    # split halves so tanh starts after the first one
                    osumT = work.tile([P, HC, P], DT, tag="osumT")
                    nc.vector.tensor_add(
                        osumT[:, 0 : HC // 2], ops_[:, 0 : HC // 2], constT2[:, 0 : HC // 2]
                    )
                    nc.vector.tensor_add(
                        osumT[:, HC // 2 :], ops_[:, HC // 2 :], constT2[:, HC // 2 :]
                    )
                    # tanh with per-partition bo bias, writing the classifier
                    # layout directly: tT_all[:, ht, io, state]
                    for ht in range(HC):
                        nc.scalar.activation(
                            tT_all[:, ht, :, 2 * pi : 2 * pi + 2],
                            osumT[:, ht, :].rearrange("p (st io) -> p io st", st=2),
                            AF.Tanh,
                            bias=boT_sb[:, ht : ht + 1],
                        )

                # ================= emission schedule =================
                nc.sync.dma_start(osT2_sb[:], osT2_d[:])
                nc.sync.dma_start(osT2b_sb[:], osT2b_d[:])
                issue_stT(0)
                nc.sync.dma_start(wob_sb[:], wo_bot_d[:])
                issue_stT(1)
                issue_sn(0)
                nc.sync.dma_start(boT_sb[:], boT_d[:])
                make_identity(nc, ident[:])
                nc.vector.tensor_add(sel[:], ident[:, 0:IO], ident[:, IO:P])

                attn_part(0, [0, 1, 2, 3])

                # constT = (output_set @ Wo_bot)^T, flipped like o; no bias
                cps = ps_o.tile([P, HC, IO], F32, tag="ps_o", name="cps")
                for ht in range(HC):
                    for hc in range(HC):
                        nc.tensor.matmul(
                            cps[:, ht, :],
                            lhsT=wob_sb[:, hc, ht * P : (ht + 1) * P],
                            rhs=osT2b_sb[:, hc, 0:IO],
                            start=(hc == 0),
                            stop=(hc == HC - 1),
                            skip_group_check=True,
                        )
                nc.vector.tensor_copy(constT2[:, :, 0:IO], cps[:])
                nc.scalar.copy(constT2[:, :, IO:P], cps[:])

                softmax(0)
                issue_sn(1)
                nc.sync.dma_start(wo_top_sb[:], wo_top_d[:])
                nc.sync.dma_start(bct_sb[:], bct_d[:])
                issue_stT(2)
                issue_sn(2)
                issue_stT(3)
                nc.sync.dma_start(wc_sb[0][:], wc_d[:, 0:NJG])

                # software-pipelined pair blocks: rest of pair pi-1 with
                # pair pi's score matmuls interleaved as PE gap fillers
                for pi in range(1, NPAIR + 1):
                    prev = pi - 1
                    atrans(prev)
                    if pi < NPAIR:
                        attn_part(pi, [0, 1])
                    attnT_copies(prev)
                    flipmix(prev)
                    if pi < NPAIR:
                        attn_part(pi, [2, 3])
                    mixT_copies(prev)
                    flipo_tanh(prev)
                    if pi < NPAIR:
                        softmax(pi)
                    if pi == 1:
                        # sn3 reuses sn0's buffer: emit only after flipmix(0)
                        issue_sn(3)
                        nc.sync.dma_start(wc_sb[1][:], wc_d[:, NJG : 2 * NJG])

                # ---- classifier, i-pair packed (valid quadrants disjoint in PSUM):
                # lhsT = [Wc_{2j} | Wc_{2j+1}] (128 cols), rhs = [t_{2j} | t_{2j+1}]
                # psum rows 0:64 accumulate even-i partial logitsT, 64:128 odd-i.
                # hc-outer so the first matmuls only need tanh's first slice.
                lgps = ps_attn.tile([P, 2 * NLOC], F32, tag="ps_attn", name="lgps")
                for hc in range(HC):
                    for jg in range(NWG):
                        for jl in range(NJG):
                            j = jg * NJG + jl
                            nc.tensor.matmul(
                                lgps[:],
                                lhsT=wc_sb[jg][:, jl, hc, :],
                                rhs=tT_all[:, hc, 2 * j : 2 * j + 2, :],
                                start=(hc == 0 and j == 0),
                                stop=(hc == HC - 1 and j == IO // 2 - 1),
                                skip_group_check=True,
                            )
                # epilogue: fold odd-i quadrant onto even via stacked-identity matmul
                lt2 = work.tile([P, NLOC], DT, tag="lt2")
                nc.vector.tensor_copy(lt2[0:IO, :], lgps[0:IO, 0:NLOC])
                nc.vector.tensor_copy(lt2[IO:P, :], lgps[IO:P, NLOC : 2 * NLOC])
                foldps = ps_attn.tile(
                    [IO, NLOC], F32, tag="ps_attn", name="foldps"
                )
                nc.tensor.matmul(foldps[:], lhsT=sel[:], rhs=lt2[:], start=True, stop=True)
                lt_sb = work.tile([IO, NLOC], F32, tag="lt_sb")
                nc.vector.tensor_add(lt_sb[:], foldps[:], bct_sb[:])
                nc.sync.dma_start(out_d[:], lt_sb[:])

    nc.compile()
    return nc


def make_in_maps(states, output_set, Wo, bo, Wc, bc):
    """Build the per-core input maps (host-side sharding + layout prep)."""
    states = np.asarray(states, dtype=np.float32)
    output_set = np.asarray(output_set, dtype=np.float32)
    Wo = np.asarray(Wo, dtype=np.float32)
    bo = np.asarray(bo, dtype=np.float32)
    Wc = np.asarray(Wc, dtype=np.float32)
    bc = np.asarray(bc, dtype=np.float32)

    osT = output_set.T  # [H, IO]
    osT2 = np.concatenate([osT, osT], axis=1).reshape(HC, P, 2 * IO).transpose(1, 0, 2)
    shared = {
        "osT2": np.ascontiguousarray(osT2).astype(NPF8),
        "osT2b": np.ascontiguousarray(osT2).astype(NPDT),
        "wo_top": np.ascontiguousarray(
            Wo[:H].reshape(HC, P, H).transpose(1, 0, 2)
        ).astype(NPDT),
        "wo_bot": np.ascontiguousarray(
            Wo[H:].reshape(HC, P, H).transpose(1, 0, 2)
        ).astype(NPDT),
        "boT": np.ascontiguousarray(bo.reshape(HC, P).T).astype(np.float32),
        # Wc[(2j+t)*H + hc*128 + hp, c] -> [hp, j, hc, t*64+c]
        "wc": np.ascontiguousarray(
            Wc.reshape(IO // 2, 2, HC, P, IO)
            .transpose(3, 0, 2, 1, 4)
            .reshape(P, IO // 2, HC, P)
        ).astype(NPDT),
        "bct": np.ascontiguousarray(np.tile(bc[:, None], (1, NLOC))).astype(
            np.float32
        ),
    }
    in_maps = []
    for k in range(NCORES):
        sl = states[k * NLOC : (k + 1) * NLOC]  # [NLOC, S, H]
        # [NPAIR, P, 2, SC, H]: [pi, p, n, sc, h] = sl[2pi+n, sc*128+p, h]
        sn_pack = sl.reshape(NPAIR, 2, SC, P, H).transpose(0, 3, 1, 2, 4)
        # [NPAIR, P, 2, HC, S]: [pi, p, n, hc, s] = sl[2pi+n, s, hc*128+p]
        stT_pack = sl.reshape(NPAIR, 2, S, HC, P).transpose(0, 4, 1, 3, 2)
        in_maps.append(
            {
                "states": np.ascontiguousarray(sn_pack).astype(NPDT),
                "statesT": np.ascontiguousarray(stT_pack).astype(NPF8),
                **shared,
            }
        )
    return in_maps


_NC_CACHE = {}


def get_nc(reps=1):
    if reps not in _NC_CACHE:
        _NC_CACHE[reps] = build_bass(reps)
    return _NC_CACHE[reps]


def kernel(states, output_set, Wo, bo, Wc, bc):
    from concourse.bass_utils import run_bass_kernel_spmd

    nc = get_nc()
    in_maps = make_in_maps(states, output_set, Wo, bo, Wc, bc)
    res = run_bass_kernel_spmd(nc, in_maps, core_ids=list(range(NCORES)))
    out = np.concatenate(
        [np.asarray(res.results[k]["logitsT"]).T for k in range(NCORES)], axis=0
    )
    return out.astype(np.float32)
